# revision 18
# baseline (speedup 1.0000x reference)
"""Trainium2 Bass kernel for nn_ClassMLP (Mamba block + MLP head).

Self-contained: hardcodes all shapes/sharding. Data-parallel over the batch
axis across 8 NeuronCores; params replicated. Inside each core everything is
feature-major ([channel, token]); x is pre-transposed on the host.

Math (per batch row, L=16, d_state=1):
  xc  = x @ Wc^T                      (in_proj, xc half; z only needed at t=15)
  conv= causal depthwise conv4(xc)+cb ; xs = silu(conv)
  dt|B|C = xs @ Wxp^T                 (dt_rank=32, B,C scalars per token)
  r   = dt @ Wdt^T + bdt ; delta = softplus(r) = ln(1+e^r)
  dA  = exp(A*delta)  (A<0 per channel) ; dBx = delta*B*xs
  h_t = dA_t*h_{t-1} + dBx_t          (tensor_tensor_scan; dA[t=0]:=0 resets)
  y   = (h_15*C_15 + D*xs_15) * silu(z_15)
  out = log_softmax(MLP(y @ Wo^T))    (BN folded into scale/bias)
"""
import os
import numpy as np

_B, _L, _DM, _DI = 8192, 16, 512, 1024
_HID, _OUT = 1024, 64
_NC = 8
_EPS = 1e-5

_BUILD_CACHE = {}
_TRACE = False
_last_exec_ns = None
_last_trace_path = None


def _build(NB, TB):
    """Build + finalize the Bass module for one core handling NB batch rows,
    processing TB tokens per block (TB multiple of 16)."""
    import concourse.mybir as mybir
    from concourse import bacc
    from concourse.tile import TileContext
    from concourse.masks import make_identity

    F32, F16 = mybir.dt.float32, mybir.dt.float16
    AF = mybir.ActivationFunctionType
    OP = mybir.AluOpType

    NTOK = NB * _L
    NBLK = NTOK // TB
    BB = TB // _L          # batch rows per block
    NBT = NB // 128        # output batch tiles
    NZ = NB // 512         # 512-wide n-tiles over NB tokens (tail)
    NN = TB // 512         # 512-wide n-tiles over TB tokens

    nc = bacc.Bacc("TRN2", target_bir_lowering=False)

    xT = nc.declare_dram_parameter("xT", [_DM, NTOK], F16, isOutput=False)
    wc = nc.declare_dram_parameter("wc", [4, 128, _DI], F16, isOutput=False)
    wz = nc.declare_dram_parameter("wz", [4, 128, _DI], F16, isOutput=False)
    xp = nc.declare_dram_parameter("xp", [8, 128, 66], F16, isOutput=False)
    dtp = nc.declare_dram_parameter("dtp", [33, _DI], F16, isOutput=False)
    opw = nc.declare_dram_parameter("opw", [8, 128, _DM], F16, isOutput=False)
    l0 = nc.declare_dram_parameter("l0", [4, 128, _HID], F16, isOutput=False)
    l1 = nc.declare_dram_parameter("l1", [8, 128, _HID], F16, isOutput=False)
    l2 = nc.declare_dram_parameter("l2", [8, 128, _OUT], F16, isOutput=False)
    # per-channel vectors, packed [part, etile, idx]:
    # 0..3 conv taps w0..w3, 4 conv_b, 5 A, 6 D_skip, 7 s0, 8 c0, 9 s1, 10 c1
    vecs = nc.declare_dram_parameter("vecs", [128, 8, 11], F32, isOutput=False)
    l2b = nc.declare_dram_parameter("l2b", [_OUT, 1], F32, isOutput=False)
    out_d = nc.declare_dram_parameter("out", [NB, _OUT], F32, isOutput=True)

    xs_sp = nc.dram_tensor("xs_spill", [8, 128, NTOK], F16)

    xT_v = xT.ap().rearrange("(a p) n -> p a n", p=128)          # [128,4,NTOK]
    xs_sp_v = xs_sp.ap().rearrange("e p n -> p e n")             # [128,8,NTOK]

    with TileContext(nc) as tc:
        with tc.tile_pool(name="persist", bufs=1) as pp:
            t_vecs = pp.tile([128, 8, 11], F32, tag="vecs")
            nc.sync.dma_start(out=t_vecs, in_=vecs.ap())
            t_l2b = pp.tile([_OUT, 1], F32, tag="l2b")
            nc.sync.dma_start(out=t_l2b, in_=l2b.ap())
            t_ones = pp.tile([1, 128], F16, tag="ones")
            nc.vector.memset(t_ones, 1.0)
            t_ident = pp.tile([128, 128], F32, tag="ident")
            make_identity(nc, t_ident)
            h15a = pp.tile([128, 8, NB], F16, tag="h15a")
            xs15a = pp.tile([128, 8, NB], F16, tag="xs15a")
            cc15 = pp.tile([1, NB], F16, tag="cc15")
            x15 = pp.tile([128, 4, NB], F16, tag="x15")

            def sv(m, i):
                return t_vecs[:, m, i:i + 1]

            # ---------------- stage 1: in_proj + conv + silu ----------------
            with (
                tc.tile_pool(name="s1w", bufs=1) as s1w,
                tc.tile_pool(name="s1", bufs=2) as s1,
                tc.tile_pool(name="ps1", bufs=3, space="PSUM") as ps1,
            ):
                t_wc = s1w.tile([128, 4, _DI], F16, tag="wc")
                nc.sync.dma_start(out=t_wc, in_=wc.ap().rearrange("k p m -> p k m"))
                for j in range(NBLK):
                    xb = s1.tile([128, 4, TB], F16, tag="xb")
                    nc.sync.dma_start(out=xb, in_=xT_v[:, :, j * TB:(j + 1) * TB])
                    nc.gpsimd.tensor_copy(
                        x15[:, :, j * BB:(j + 1) * BB],
                        xb.rearrange("p a (g t) -> p a g t", t=_L)[:, :, :, 15])
                    xc = s1.tile([128, 8, TB], F16, tag="xc")
                    acc = s1.tile([128, 8, TB], F16, tag="acc")
                    for m in range(8):
                        pm = ps1.tile([128, TB], F32, tag="mm")
                        for n in range(NN):
                            ns = slice(n * 512, (n + 1) * 512)
                            for k in range(4):
                                nc.tensor.matmul(
                                    pm[:, ns],
                                    t_wc[:, k, m * 128:(m + 1) * 128],
                                    xb[:, k, ns],
                                    start=(k == 0), stop=(k == 3),
                                )
                        nc.vector.tensor_copy(xc[:, m, :], pm)
                        # conv tap3 (aligned) + conv bias
                        nc.vector.tensor_scalar(
                            acc[:, m, :], xc[:, m, :], sv(m, 3), sv(m, 4),
                            OP.mult, OP.add)
                    for m in range(8):
                        accg = acc[:, m, :].rearrange("p (g t) -> p g t", t=_L)
                        xcg = xc[:, m, :].rearrange("p (g t) -> p g t", t=_L)
                        for k in (2, 1, 0):
                            s = 3 - k
                            nc.vector.scalar_tensor_tensor(
                                accg[:, :, s:], xcg[:, :, :_L - s], sv(m, k),
                                accg[:, :, s:], OP.mult, OP.add)
                    xs = s1.tile([128, 8, TB], F16, tag="xs")
                    nc.scalar.activation(xs, acc, AF.Silu)
                    nc.sync.dma_start(
                        out=xs_sp_v[:, :, j * TB:(j + 1) * TB], in_=xs)
                    nc.vector.tensor_copy(
                        xs15a[:, :, j * BB:(j + 1) * BB],
                        xs.rearrange("p e (g t) -> p e g t", t=_L)[:, :, :, 15])

            # ---------------- stage 2: x_proj/dt_proj + scan ----------------
            with (
                tc.tile_pool(name="s2w", bufs=1) as s2w,
                tc.tile_pool(name="s2", bufs=1) as s2,
                tc.tile_pool(name="s2b", bufs=1) as s2b,
                tc.tile_pool(name="ps2", bufs=2, space="PSUM") as ps2,
                tc.tile_pool(name="psx", bufs=1, space="PSUM") as psx,
                tc.tile_pool(name="pst", bufs=1, space="PSUM") as pst,
            ):
                t_xp = s2w.tile([128, 8, 66], F16, tag="xp")
                nc.sync.dma_start(out=t_xp, in_=xp.ap().rearrange("k p m -> p k m"))
                t_dtp = s2w.tile([33, _DI], F16, tag="dtp")
                nc.sync.dma_start(out=t_dtp, in_=dtp.ap())

                for j in range(NBLK):
                    xs = s2b.tile([128, 8, TB], F16, tag="xs2")
                    nc.sync.dma_start(
                        out=xs, in_=xs_sp_v[:, :, j * TB:(j + 1) * TB])
                    # x_proj -> [66, TB] (dt 0..31, B at 32, C at 64)
                    pxd = psx.tile([66, TB], F32, tag="xd")
                    for n in range(NN):
                        ns = slice(n * 512, (n + 1) * 512)
                        for k in range(8):
                            nc.tensor.matmul(
                                pxd[:, ns], t_xp[:, k, :], xs[:, k, ns],
                                start=(k == 0), stop=(k == 7))
                    dts = s2.tile([33, TB], F16, tag="dts")
                    nc.vector.tensor_copy(dts[0:32, :], pxd[0:32, :])
                    nc.gpsimd.memset(dts[32:33, :], 1.0)
                    bc = s2.tile([1, TB], F16, tag="bc")
                    nc.vector.tensor_copy(bc, pxd[32:33, :])
                    nc.vector.tensor_copy(
                        cc15[:, j * BB:(j + 1) * BB],
                        pxd[64:65, :].rearrange("p (g t) -> p g t", t=_L)[:, :, 15])
                    # broadcast B across partitions via ones outer-product
                    pbc = ps2.tile([128, TB], F32, tag="mm")
                    for n in range(NN):
                        ns = slice(n * 512, (n + 1) * 512)
                        nc.tensor.matmul(pbc[:, ns], t_ones, bc[0:1, ns],
                                         start=True, stop=True)
                    bcb = s2.tile([128, TB], F16, tag="bcb")
                    nc.vector.tensor_copy(bcb, pbc)
                    # dt_proj (K=33 incl. bias row) -> exp -> u
                    u = s2.tile([128, 8, TB], F16, tag="u")
                    for m in range(8):
                        pr = ps2.tile([128, TB], F32, tag="mm")
                        for n in range(NN):
                            ns = slice(n * 512, (n + 1) * 512)
                            nc.tensor.matmul(
                                pr[:, ns], t_dtp[:, m * 128:(m + 1) * 128],
                                dts[:, ns], start=True, stop=True)
                        nc.scalar.activation(u[:, m, :], pr, AF.Exp)
                    delta = s2.tile([128, 8, TB], F16, tag="delta")
                    nc.scalar.activation(delta, u, AF.Ln, bias=1.0)
                    da = s2.tile([128, 8, TB], F16, tag="da")
                    for m in range(8):
                        nc.scalar.activation(da[:, m, :], delta[:, m, :],
                                             AF.Exp, scale=sv(m, 5))
                    nc.gpsimd.memset(
                        da.rearrange("p e (g t) -> p e g t", t=_L)[:, :, :, 0:1],
                        0.0)
                    P = s2.tile([128, 8, TB], F16, tag="u")      # reuse u slot
                    nc.vector.tensor_tensor(P, delta, xs, OP.mult)
                    dbx = s2.tile([128, 8, TB], F16, tag="delta")  # reuse
                    for m in range(8):
                        nc.vector.tensor_tensor(dbx[:, m, :], P[:, m, :], bcb,
                                                OP.mult)
                    h = s2.tile([128, 8, TB], F16, tag="h")
                    for m in range(8):
                        nc.vector.tensor_tensor_scan(
                            h[:, m, :], da[:, m, :], dbx[:, m, :], 0.0,
                            OP.mult, OP.add)
                    nc.vector.tensor_copy(
                        h15a[:, :, j * BB:(j + 1) * BB],
                        h.rearrange("p e (g t) -> p e g t", t=_L)[:, :, :, 15])

                # ---------------- tail: t=15 only ----------------
                t_wz = s2w.tile([128, 4, _DI], F16, tag="wz")
                nc.sync.dma_start(out=t_wz, in_=wz.ap().rearrange("k p m -> p k m"))
                t_opw = s2w.tile([128, 8, _DM], F16, tag="opw")
                nc.sync.dma_start(out=t_opw, in_=opw.ap().rearrange("k p m -> p k m"))
                t_l0 = s2w.tile([128, 4, _HID], F16, tag="l0")
                nc.sync.dma_start(out=t_l0, in_=l0.ap().rearrange("k p m -> p k m"))
                t_l1 = s2w.tile([128, 8, _HID], F16, tag="l1")
                nc.sync.dma_start(out=t_l1, in_=l1.ap().rearrange("k p m -> p k m"))
                t_l2 = s2w.tile([128, 8, _OUT], F16, tag="l2")
                nc.sync.dma_start(out=t_l2, in_=l2.ap().rearrange("k p m -> p k m"))

                sz = s2.tile([128, 8, NB], F16, tag="u")
                for m in range(8):
                    pz = ps2.tile([128, NB], F32, tag="mm")
                    for n in range(NZ):
                        ns = slice(n * 512, (n + 1) * 512)
                        for k in range(4):
                            nc.tensor.matmul(
                                pz[:, ns], t_wz[:, k, m * 128:(m + 1) * 128],
                                x15[:, k, ns], start=(k == 0), stop=(k == 3))
                    nc.scalar.activation(sz[:, m, :], pz, AF.Silu)
                # broadcast C_15
                pcc = ps2.tile([128, NB], F32, tag="mm")
                for n in range(NZ):
                    ns = slice(n * 512, (n + 1) * 512)
                    nc.tensor.matmul(pcc[:, ns], t_ones, cc15[:, ns],
                                     start=True, stop=True)
                ccb = s2.tile([128, NB], F16, tag="ccb")
                nc.vector.tensor_copy(ccb, pcc)
                # y = (h15*C + D*xs15) * silu(z15)
                ym = s2.tile([128, 8, NB], F16, tag="h")
                for m in range(8):
                    t1 = s2.tile([128, NB], F16, tag="bcb")
                    nc.vector.tensor_tensor(t1, h15a[:, m, :], ccb, OP.mult)
                    nc.vector.scalar_tensor_tensor(
                        t1, xs15a[:, m, :], sv(m, 6), t1, OP.mult, OP.add)
                    nc.vector.tensor_tensor(ym[:, m, :], t1, sz[:, m, :],
                                            OP.mult)
                # out_proj -> o [512, NB]
                o_sb = s2.tile([128, 4, NB], F16, tag="o_sb")
                for dm in range(4):
                    po = ps2.tile([128, NB], F32, tag="mm")
                    for n in range(NZ):
                        ns = slice(n * 512, (n + 1) * 512)
                        for k in range(8):
                            nc.tensor.matmul(
                                po[:, ns], t_opw[:, k, dm * 128:(dm + 1) * 128],
                                ym[:, k, ns], start=(k == 0), stop=(k == 7))
                    nc.vector.tensor_copy(o_sb[:, dm, :], po)
                # lin0 + bn0 + relu
                h1 = s2.tile([128, 8, NB], F16, tag="da")   # reuse after y
                for m in range(8):
                    pl = ps2.tile([128, NB], F32, tag="mm")
                    for n in range(NZ):
                        ns = slice(n * 512, (n + 1) * 512)
                        for k in range(4):
                            nc.tensor.matmul(
                                pl[:, ns], t_l0[:, k, m * 128:(m + 1) * 128],
                                o_sb[:, k, ns], start=(k == 0), stop=(k == 3))
                    nc.scalar.activation(h1[:, m, :], pl, AF.Relu,
                                         scale=sv(m, 7), bias=sv(m, 8))
                # lin1 + bn1 + relu
                h2 = s2.tile([128, 8, NB], F16, tag="delta")   # reuse
                for m in range(8):
                    pl = ps2.tile([128, NB], F32, tag="mm")
                    for n in range(NZ):
                        ns = slice(n * 512, (n + 1) * 512)
                        for k in range(8):
                            nc.tensor.matmul(
                                pl[:, ns], t_l1[:, k, m * 128:(m + 1) * 128],
                                h1[:, k, ns], start=(k == 0), stop=(k == 7))
                    nc.scalar.activation(h2[:, m, :], pl, AF.Relu,
                                         scale=sv(m, 9), bias=sv(m, 10))
                # lin2 -> logits [64, NB] fp32 (+bias)
                lg = s2.tile([_OUT, NB], F32, tag="lg")
                plg = psx.tile([_OUT, NB], F32, tag="xd")
                for n in range(NZ):
                    ns = slice(n * 512, (n + 1) * 512)
                    for k in range(8):
                        nc.tensor.matmul(plg[:, ns], t_l2[:, k, :],
                                         h2[:, k, ns], start=(k == 0),
                                         stop=(k == 7))
                nc.vector.tensor_scalar(lg, plg, t_l2b, None, OP.add)
                # log_softmax per 128-row tile, output token-major
                for bt in range(NBT):
                    pt = pst.tile([128, _OUT], F32, tag="tr")
                    nc.tensor.transpose(
                        pt, lg[:, bt * 128:(bt + 1) * 128],
                        t_ident[0:_OUT, 0:_OUT])
                    nmx = s2.tile([128, 1], F32, tag="nmx")
                    nc.vector.tensor_reduce(nmx, pt, mybir.AxisListType.X,
                                            OP.max, negate=True)
                    es = s2.tile([128, _OUT], F32, tag="es")
                    se = s2.tile([128, 1], F32, tag="se")
                    nc.scalar.activation(es, pt, AF.Exp, bias=nmx,
                                         accum_out=se)
                    lnse = s2.tile([128, 1], F32, tag="lnse")
                    nc.scalar.activation(lnse, se, AF.Ln)
                    ls = s2.tile([128, _OUT], F32, tag="ls")
                    nc.vector.tensor_scalar(ls, pt, nmx, lnse, OP.add,
                                            OP.subtract)
                    nc.sync.dma_start(
                        out=out_d.ap()[bt * 128:(bt + 1) * 128, :], in_=ls)

    nc.finalize()
    return nc


def _prep_inputs(x, in_proj_w, conv_w, conv_b, x_proj_w, dt_proj_w, dt_proj_b,
                 A_log, D_skip, out_proj_w, lin0_w, lin0_b, lin1_w, lin1_b,
                 lin2_w, lin2_b, bn0_g, bn0_b, bn0_m, bn0_v, bn1_g, bn1_b,
                 bn1_m, bn1_v, NB):
    f16 = np.float16
    f32 = np.float32

    def pack_vec(v):
        return np.ascontiguousarray(np.asarray(v, f32).reshape(8, 128).T)

    wc_ = np.ascontiguousarray(
        np.asarray(in_proj_w[:_DI], f32).T.reshape(4, 128, _DI)).astype(f16)
    wz_ = np.ascontiguousarray(
        np.asarray(in_proj_w[_DI:], f32).T.reshape(4, 128, _DI)).astype(f16)
    xpm = np.zeros((_DI, 66), f32)
    xpm[:, :32] = np.asarray(x_proj_w, f32)[:32].T
    xpm[:, 32] = np.asarray(x_proj_w, f32)[32]
    xpm[:, 64] = np.asarray(x_proj_w, f32)[33]
    xp_ = np.ascontiguousarray(xpm.reshape(8, 128, 66)).astype(f16)
    dtp_ = np.concatenate(
        [np.asarray(dt_proj_w, f32).T,
         np.asarray(dt_proj_b, f32)[None, :]], axis=0).astype(f16)  # [33,1024]
    opw_ = np.ascontiguousarray(
        np.asarray(out_proj_w, f32).T.reshape(8, 128, _DM)).astype(f16)
    l0_ = np.ascontiguousarray(
        np.asarray(lin0_w, f32).T.reshape(4, 128, _HID)).astype(f16)
    l1_ = np.ascontiguousarray(
        np.asarray(lin1_w, f32).T.reshape(8, 128, _HID)).astype(f16)
    l2_ = np.ascontiguousarray(
        np.asarray(lin2_w, f32).T.reshape(8, 128, _OUT)).astype(f16)

    s0 = np.asarray(bn0_g, f32) / np.sqrt(np.asarray(bn0_v, f32) + _EPS)
    c0 = s0 * (np.asarray(lin0_b, f32) - np.asarray(bn0_m, f32)) + np.asarray(bn0_b, f32)
    s1 = np.asarray(bn1_g, f32) / np.sqrt(np.asarray(bn1_v, f32) + _EPS)
    c1 = s1 * (np.asarray(lin1_b, f32) - np.asarray(bn1_m, f32)) + np.asarray(bn1_b, f32)
    A = -np.exp(np.asarray(A_log, f32)[:, 0])

    vecs = np.stack(
        [pack_vec(np.asarray(conv_w, f32)[:, k]) for k in range(4)]
        + [pack_vec(conv_b), pack_vec(A), pack_vec(D_skip),
           pack_vec(s0), pack_vec(c0), pack_vec(s1), pack_vec(c1)],
        axis=2)  # [128, 8, 11]
    vecs = np.ascontiguousarray(vecs, f32)
    l2b_ = np.ascontiguousarray(np.asarray(lin2_b, f32)[:, None])

    shared = dict(wc=wc_, wz=wz_, xp=xp_, dtp=dtp_, opw=opw_, l0=l0_, l1=l1_,
                  l2=l2_, vecs=vecs, l2b=l2b_)

    n_shards = np.asarray(x).shape[0] // NB
    in_maps = []
    for c in range(n_shards):
        xs_ = np.asarray(x[c * NB:(c + 1) * NB], f32).reshape(NB * _L, _DM)
        xT = np.ascontiguousarray(xs_.T).astype(f16)
        in_maps.append(dict(shared, xT=xT))
    return in_maps


def kernel(**inputs):
    from concourse.bass_utils import run_bass_kernel_spmd

    NB = _B // _NC
    TB = 1024
    key = (NB, TB)
    if key not in _BUILD_CACHE:
        _BUILD_CACHE[key] = _build(NB, TB)
    nc = _BUILD_CACHE[key]

    in_maps = _prep_inputs(NB=NB, **inputs)
    res = run_bass_kernel_spmd(nc, in_maps, core_ids=list(range(_NC)),
                               trace=_TRACE)
    global _last_exec_ns, _last_trace_path
    _last_exec_ns = res.exec_time_ns
    if res.instructions_and_trace:
        _last_trace_path = res.instructions_and_trace[1]
    out = np.concatenate([r["out"] for r in res.results], axis=0)
    return out.astype(np.float32)


# revision 26
# speedup vs baseline: 3816.3227x; 3816.3227x over previous
"""Trainium2 Bass kernel for nn_ClassMLP (Mamba block + MLP head).

Self-contained: hardcodes all shapes/sharding. Data-parallel over the batch
axis across 8 NeuronCores; params replicated. Inside each core everything is
feature-major ([channel, token]); x is pre-transposed on the host.

Math (per batch row, L=16, d_state=1):
  xc  = x @ Wc^T                      (in_proj xc half; z only needed at t=15)
  conv= causal depthwise conv4(xc)+cb ; xs = silu(conv)
  dt|B|C = xs @ Wxp^T                 (dt_rank=32; B,C scalars per token)
  r   = dt @ Wdt^T + bdt ; delta = softplus(r) = ln(1+e^r)
  dA  = exp(A*delta)  (A<0 per channel) ; dBx = delta*B*xs
  h_t = dA_t*h_{t-1} + dBx_t          (tensor_tensor_scan; dA[t=0]:=0 resets)
  y   = (h_15*C_15 + D*xs_15) * silu(z_15)
  out = log_softmax(MLP(y @ Wo^T))    (BN folded into scale/bias)

Conv trick: the in_proj weights for xc are pre-scaled by tap3 (w3) on the
host, so PSUM holds w3*xc. Tap k (k<3) is then r_k*(acc-cb) with
r_k = w_k/w3 — a 4x-mode tensor_scalar with two scalar operands — followed
by a 2x-mode in-place shifted tensor_tensor add. This keeps the depthwise
conv entirely in fast DVE modes (scalar_tensor_tensor has no 2x/4x uops).
"""
import numpy as np

_B, _L, _DM, _DI = 8192, 16, 512, 1024
_HID, _OUT = 1024, 64
_NC = 8
_EPS = 1e-5

_BUILD_CACHE = {}
# tuning knobs
_ACT_X = 8          # of 8 e-tiles, how many psum->sbuf extracts go to ACT
_SCAN_ON_GPSIMD = 0  # of 8 scan calls per block, how many go to gpsimd


def _make_bacc():
    import concourse.mybir as mybir
    from concourse import bacc
    from concourse.hw_specs import get_activation_tables
    import bass_rust as _bass_rust

    class _Bacc(bacc.Bacc):
        """Bacc that restricts ACT table-set choice to the two sets we batch
        around (combined exp+ln set, and silu set), so the table-load pass
        doesn't alternate exp_and_others/natural_log every block."""

        def insert_act_table_loads(self):
            has_activation = any(
                isinstance(i, mybir.InstActivation)
                for b in self.main_func.blocks
                for i in b.instructions
            )
            if not has_activation:
                return
            keep = {"natural_log_exp_and_others", "silu_and_others"}
            tables = [
                (name, (funcs if name in keep else set()))
                for name, funcs in get_activation_tables(self.m.arch).items()
            ]
            _bass_rust.insert_act_table_loads(self, tables)

    return _Bacc("TRN2", target_bir_lowering=False)


def _build(NB, TB):
    """Build + finalize the Bass module for one core handling NB batch rows,
    processing TB tokens per block (TB multiple of 16, NB multiple of 512)."""
    import concourse.mybir as mybir
    from concourse.tile import TileContext
    from concourse.masks import make_identity

    F32, F16 = mybir.dt.float32, mybir.dt.float16
    AF = mybir.ActivationFunctionType
    OP = mybir.AluOpType

    NTOK = NB * _L
    NBLK = NTOK // TB
    BB = TB // _L          # batch rows per block
    NBT = NB // 128        # output batch tiles
    NZ = NB // 512         # 512-wide n-tiles over NB (tail)
    NN = TB // 512         # 512-wide n-tiles over TB

    nc = _make_bacc()

    xT = nc.declare_dram_parameter("xT", [_DM, NTOK], F16, isOutput=False)
    wc = nc.declare_dram_parameter("wc", [4, 128, _DI], F16, isOutput=False)
    wz = nc.declare_dram_parameter("wz", [4, 128, _DI], F16, isOutput=False)
    xp = nc.declare_dram_parameter("xp", [8, 128, 66], F16, isOutput=False)
    dtp = nc.declare_dram_parameter("dtp", [33, _DI], F16, isOutput=False)
    opw = nc.declare_dram_parameter("opw", [8, 128, _DM], F16, isOutput=False)
    l0 = nc.declare_dram_parameter("l0", [4, 128, _HID], F16, isOutput=False)
    l1 = nc.declare_dram_parameter("l1", [8, 128, _HID], F16, isOutput=False)
    l2 = nc.declare_dram_parameter("l2", [8, 128, _OUT], F16, isOutput=False)
    # per-channel vectors, packed [part, etile, idx]:
    # 0..2 tap ratios r_k=w_k/w3, 3..5 -r_k*cb, 6 cb, 7 A, 8 D_skip,
    # 9 s0, 10 c0, 11 s1, 12 c1
    vecs = nc.declare_dram_parameter("vecs", [128, 8, 13], F32, isOutput=False)
    l2b = nc.declare_dram_parameter("l2b", [_OUT, 1], F32, isOutput=False)
    out_d = nc.declare_dram_parameter("out", [NB, _OUT], F32, isOutput=True)

    xs_sp = nc.dram_tensor("xs_spill", [8, 128, NTOK], F16)

    xT_v = xT.ap().rearrange("(a p) n -> p a n", p=128)          # [128,4,NTOK]
    xs_sp_v = xs_sp.ap().rearrange("e p n -> p e n")             # [128,8,NTOK]

    with TileContext(nc) as tc:
        with tc.tile_pool(name="persist", bufs=1) as pp:
            t_vecs = pp.tile([128, 8, 13], F32, tag="vecs")
            nc.sync.dma_start(out=t_vecs, in_=vecs.ap())
            t_l2b = pp.tile([_OUT, 1], F32, tag="l2b")
            nc.sync.dma_start(out=t_l2b, in_=l2b.ap())
            t_ones = pp.tile([1, 128], F16, tag="ones")
            nc.vector.memset(t_ones, 1.0)
            t_ident = pp.tile([128, 128], F32, tag="ident")
            make_identity(nc, t_ident)
            h15a = pp.tile([128, 8, NB], F16, tag="h15a")
            xs15a = pp.tile([128, 8, NB], F16, tag="xs15a")
            cc15 = pp.tile([1, NB], F16, tag="cc15")
            x15 = pp.tile([128, 4, NB], F16, tag="x15")

            def sv(m, i):
                return t_vecs[:, m, i:i + 1]

            # ---------------- stage 1: in_proj + conv + silu ----------------
            with (
                tc.tile_pool(name="s1w", bufs=1) as s1w,
                tc.tile_pool(name="s1", bufs=2) as s1,
                tc.tile_pool(name="s1t", bufs=2) as s1t,
                tc.tile_pool(name="ps1", bufs=3, space="PSUM") as ps1,
            ):
                t_wc = s1w.tile([128, 4, _DI], F16, tag="wc")
                nc.sync.dma_start(out=t_wc, in_=wc.ap().rearrange("k p m -> p k m"))
                for j in range(NBLK):
                    xb = s1.tile([128, 4, TB], F16, tag="xb")
                    nc.sync.dma_start(out=xb, in_=xT_v[:, :, j * TB:(j + 1) * TB])
                    nc.gpsimd.tensor_copy(
                        x15[:, :, j * BB:(j + 1) * BB],
                        xb.rearrange("p a (g t) -> p a g t", t=_L)[:, :, :, 15])
                    acc = s1.tile([128, 8, TB], F16, tag="acc")
                    for m in range(8):
                        pm = ps1.tile([128, TB], F32, tag="mm")
                        for n in range(NN):
                            ns = slice(n * 512, (n + 1) * 512)
                            for k in range(4):
                                nc.tensor.matmul(
                                    pm[:, ns],
                                    t_wc[:, k, m * 128:(m + 1) * 128],
                                    xb[:, k, ns],
                                    start=(k == 0), stop=(k == 3),
                                )
                        # extract + conv bias: acc_m = w3*xc + cb
                        if m < _ACT_X:
                            nc.scalar.activation(acc[:, m, :], pm, AF.Identity,
                                                 bias=sv(m, 6))
                        else:
                            nc.vector.tensor_scalar(acc[:, m, :], pm, sv(m, 6),
                                                    None, OP.add)
                    for m in range(8):
                        # taps k=0..2: t_k = r_k*acc - r_k*cb  (= w_k*xc), 4x
                        tks = []
                        for k in range(3):
                            tk = s1t.tile([128, TB], F16, tag=f"tk{k}")
                            nc.vector.tensor_scalar(
                                tk, acc[:, m, :], sv(m, k), sv(m, 3 + k),
                                OP.mult, OP.add)
                            tks.append(tk)
                        accg = acc[:, m, :].rearrange("p (g t) -> p g t", t=_L)
                        for k in range(3):
                            s = 3 - k
                            tkg = tks[k].rearrange("p (g t) -> p g t", t=_L)
                            nc.vector.tensor_tensor(
                                accg[:, :, s:], accg[:, :, s:],
                                tkg[:, :, :_L - s], OP.add)
                    xs = s1.tile([128, 8, TB], F16, tag="xs")
                    nc.scalar.activation(xs, acc, AF.Silu)
                    nc.sync.dma_start(
                        out=xs_sp_v[:, :, j * TB:(j + 1) * TB], in_=xs)
                    nc.gpsimd.tensor_copy(
                        xs15a[:, :, j * BB:(j + 1) * BB],
                        xs.rearrange("p e (g t) -> p e g t", t=_L)[:, :, :, 15])

            # ---------------- stage 2: x_proj/dt_proj + scan ----------------
            with (
                tc.tile_pool(name="s2w", bufs=1) as s2w,
                tc.tile_pool(name="s2", bufs=1) as s2,
                tc.tile_pool(name="ps2", bufs=2, space="PSUM") as ps2,
                tc.tile_pool(name="psx", bufs=1, space="PSUM") as psx,
                tc.tile_pool(name="pst", bufs=1, space="PSUM") as pst,
            ):
                t_xp = s2w.tile([128, 8, 66], F16, tag="xp")
                nc.sync.dma_start(out=t_xp, in_=xp.ap().rearrange("k p m -> p k m"))
                t_dtp = s2w.tile([33, _DI], F16, tag="dtp")
                nc.sync.dma_start(out=t_dtp, in_=dtp.ap())

                with tc.tile_pool(name="s2b", bufs=2) as s2b:
                    for j in range(NBLK):
                        xs = s2b.tile([128, 8, TB], F16, tag="xs2")
                        nc.sync.dma_start(
                            out=xs, in_=xs_sp_v[:, :, j * TB:(j + 1) * TB])
                        # x_proj -> [66, TB] (dt 0..31, B at 32, C at 64)
                        pxd = psx.tile([66, TB], F32, tag="xd")
                        for n in range(NN):
                            ns = slice(n * 512, (n + 1) * 512)
                            for k in range(8):
                                nc.tensor.matmul(
                                    pxd[:, ns], t_xp[:, k, :], xs[:, k, ns],
                                    start=(k == 0), stop=(k == 7))
                        dts = s2.tile([33, TB], F16, tag="dts")
                        nc.vector.tensor_copy(dts[0:32, :], pxd[0:32, :])
                        nc.gpsimd.memset(dts[32:33, :], 1.0)
                        bc = s2.tile([1, TB], F16, tag="bc")
                        nc.vector.tensor_copy(bc, pxd[32:33, :])
                        nc.vector.tensor_copy(
                            cc15[:, j * BB:(j + 1) * BB],
                            pxd[64:65, :].rearrange("p (g t) -> p g t", t=_L)[:, :, 15])
                        # broadcast B across partitions via ones outer-product
                        pbc = ps2.tile([128, TB], F32, tag="mm")
                        for n in range(NN):
                            ns = slice(n * 512, (n + 1) * 512)
                            nc.tensor.matmul(pbc[:, ns], t_ones, bc[0:1, ns],
                                             start=True, stop=True)
                        bcb = s2.tile([128, TB], F16, tag="bcb")
                        nc.vector.tensor_copy(bcb, pbc)
                        # dt_proj (K=33 incl. bias row) -> exp -> u
                        u = s2.tile([128, 8, TB], F16, tag="u")
                        for m in range(8):
                            pr = ps2.tile([128, TB], F32, tag="mm")
                            for n in range(NN):
                                ns = slice(n * 512, (n + 1) * 512)
                                nc.tensor.matmul(
                                    pr[:, ns], t_dtp[:, m * 128:(m + 1) * 128],
                                    dts[:, ns], start=True, stop=True)
                            nc.scalar.activation(u[:, m, :], pr, AF.Exp)
                        delta = s2.tile([128, 8, TB], F16, tag="delta")
                        nc.scalar.activation(delta, u, AF.Ln, bias=1.0)
                        # prescale A*delta on DVE (4x), then one big exp
                        dp = s2.tile([128, 8, TB], F16, tag="dbx")
                        for m in range(8):
                            nc.vector.tensor_scalar(dp[:, m, :], delta[:, m, :],
                                                    sv(m, 7), None, OP.mult)
                        da = s2.tile([128, 8, TB], F16, tag="da")
                        nc.scalar.activation(da, dp, AF.Exp)
                        nc.gpsimd.memset(
                            da.rearrange("p e (g t) -> p e g t", t=_L)[:, :, :, 0:1],
                            0.0)
                        P = s2.tile([128, 8, TB], F16, tag="P")
                        nc.vector.tensor_tensor(P, delta, xs, OP.mult)
                        dbx = s2.tile([128, 8, TB], F16, tag="dbx")
                        for m in range(8):
                            nc.vector.tensor_tensor(dbx[:, m, :], P[:, m, :],
                                                    bcb, OP.mult)
                        h = s2.tile([128, 8, TB], F16, tag="h")
                        for m in range(8):
                            eng = nc.gpsimd if m < _SCAN_ON_GPSIMD else nc.vector
                            eng.tensor_tensor_scan(
                                h[:, m, :], da[:, m, :], dbx[:, m, :], 0.0,
                                OP.mult, OP.add)
                        nc.gpsimd.tensor_copy(
                            h15a[:, :, j * BB:(j + 1) * BB],
                            h.rearrange("p e (g t) -> p e g t", t=_L)[:, :, :, 15])

                # ---------------- tail: t=15 only ----------------
                with tc.tile_pool(name="s2tw", bufs=1) as s2tw:
                    t_wz = s2tw.tile([128, 4, _DI], F16, tag="wz")
                    nc.sync.dma_start(out=t_wz, in_=wz.ap().rearrange("k p m -> p k m"))
                    t_opw = s2tw.tile([128, 8, _DM], F16, tag="opw")
                    nc.sync.dma_start(out=t_opw, in_=opw.ap().rearrange("k p m -> p k m"))
                    t_l0 = s2tw.tile([128, 4, _HID], F16, tag="l0")
                    nc.sync.dma_start(out=t_l0, in_=l0.ap().rearrange("k p m -> p k m"))
                    t_l1 = s2tw.tile([128, 8, _HID], F16, tag="l1")
                    nc.sync.dma_start(out=t_l1, in_=l1.ap().rearrange("k p m -> p k m"))
                    t_l2 = s2tw.tile([128, 8, _OUT], F16, tag="l2")
                    nc.sync.dma_start(out=t_l2, in_=l2.ap().rearrange("k p m -> p k m"))

                    sz = s2.tile([128, 8, NB], F16, tag="u")
                    for m in range(8):
                        pz = ps2.tile([128, NB], F32, tag="mm")
                        for n in range(NZ):
                            ns = slice(n * 512, (n + 1) * 512)
                            for k in range(4):
                                nc.tensor.matmul(
                                    pz[:, ns], t_wz[:, k, m * 128:(m + 1) * 128],
                                    x15[:, k, ns], start=(k == 0), stop=(k == 3))
                        nc.scalar.activation(sz[:, m, :], pz, AF.Silu)
                    # broadcast C_15
                    pcc = ps2.tile([128, NB], F32, tag="mm")
                    for n in range(NZ):
                        ns = slice(n * 512, (n + 1) * 512)
                        nc.tensor.matmul(pcc[:, ns], t_ones, cc15[:, ns],
                                         start=True, stop=True)
                    ccb = s2.tile([128, NB], F16, tag="dbx")
                    nc.vector.tensor_copy(ccb, pcc)
                    # y = (h15*C + D*xs15) * silu(z15)
                    ym = s2.tile([128, 8, NB], F16, tag="h")
                    for m in range(8):
                        t1 = s2.tile([128, NB], F16, tag="bcb")
                        nc.vector.tensor_tensor(t1, h15a[:, m, :], ccb, OP.mult)
                        nc.vector.scalar_tensor_tensor(
                            t1, xs15a[:, m, :], sv(m, 8), t1, OP.mult, OP.add)
                        nc.vector.tensor_tensor(ym[:, m, :], t1, sz[:, m, :],
                                                OP.mult)
                    # out_proj -> o [512, NB]
                    o_sb = s2.tile([128, 4, NB], F16, tag="P")
                    for dm in range(4):
                        po = ps2.tile([128, NB], F32, tag="mm")
                        for n in range(NZ):
                            ns = slice(n * 512, (n + 1) * 512)
                            for k in range(8):
                                nc.tensor.matmul(
                                    po[:, ns], t_opw[:, k, dm * 128:(dm + 1) * 128],
                                    ym[:, k, ns], start=(k == 0), stop=(k == 7))
                        nc.vector.tensor_copy(o_sb[:, dm, :], po)
                    # lin0 + bn0 + relu
                    h1 = s2.tile([128, 8, NB], F16, tag="da")
                    for m in range(8):
                        pl = ps2.tile([128, NB], F32, tag="mm")
                        for n in range(NZ):
                            ns = slice(n * 512, (n + 1) * 512)
                            for k in range(4):
                                nc.tensor.matmul(
                                    pl[:, ns], t_l0[:, k, m * 128:(m + 1) * 128],
                                    o_sb[:, k, ns], start=(k == 0), stop=(k == 3))
                        nc.scalar.activation(h1[:, m, :], pl, AF.Relu,
                                             scale=sv(m, 9), bias=sv(m, 10))
                    # lin1 + bn1 + relu
                    h2 = s2.tile([128, 8, NB], F16, tag="delta")
                    for m in range(8):
                        pl = ps2.tile([128, NB], F32, tag="mm")
                        for n in range(NZ):
                            ns = slice(n * 512, (n + 1) * 512)
                            for k in range(8):
                                nc.tensor.matmul(
                                    pl[:, ns], t_l1[:, k, m * 128:(m + 1) * 128],
                                    h1[:, k, ns], start=(k == 0), stop=(k == 7))
                        nc.scalar.activation(h2[:, m, :], pl, AF.Relu,
                                             scale=sv(m, 11), bias=sv(m, 12))
                    # lin2 -> logits [64, NB] fp32 (+bias)
                    lg = s2.tile([_OUT, NB], F32, tag="bc")
                    plg = psx.tile([_OUT, NB], F32, tag="xd")
                    for n in range(NZ):
                        ns = slice(n * 512, (n + 1) * 512)
                        for k in range(8):
                            nc.tensor.matmul(plg[:, ns], t_l2[:, k, :],
                                             h2[:, k, ns], start=(k == 0),
                                             stop=(k == 7))
                    nc.vector.tensor_scalar(lg, plg, t_l2b, None, OP.add)
                    # log_softmax per 128-row tile, output token-major
                    for bt in range(NBT):
                        pt = pst.tile([128, _OUT], F32, tag="tr")
                        nc.tensor.transpose(
                            pt, lg[:, bt * 128:(bt + 1) * 128],
                            t_ident[0:_OUT, 0:_OUT])
                        nmx = s2.tile([128, 1], F32, tag="nmx")
                        nc.vector.tensor_reduce(nmx, pt, mybir.AxisListType.X,
                                                OP.max, negate=True)
                        es = s2.tile([128, _OUT], F32, tag="es")
                        se = s2.tile([128, 1], F32, tag="se")
                        nc.scalar.activation(es, pt, AF.Exp, bias=nmx,
                                             accum_out=se)
                        lnse = s2.tile([128, 1], F32, tag="lnse")
                        nc.scalar.activation(lnse, se, AF.Ln)
                        ls = s2.tile([128, _OUT], F32, tag="ls")
                        nc.vector.tensor_scalar(ls, pt, nmx, lnse, OP.add,
                                                OP.subtract)
                        nc.sync.dma_start(
                            out=out_d.ap()[bt * 128:(bt + 1) * 128, :], in_=ls)

    nc.finalize()
    return nc


def _prep_inputs(x, in_proj_w, conv_w, conv_b, x_proj_w, dt_proj_w, dt_proj_b,
                 A_log, D_skip, out_proj_w, lin0_w, lin0_b, lin1_w, lin1_b,
                 lin2_w, lin2_b, bn0_g, bn0_b, bn0_m, bn0_v, bn1_g, bn1_b,
                 bn1_m, bn1_v, NB):
    f16 = np.float16
    f32 = np.float32

    def pack_vec(v):
        return np.ascontiguousarray(np.asarray(v, f32).reshape(8, 128).T)

    cw = np.asarray(conv_w, f32)
    cb = np.asarray(conv_b, f32)
    w3 = cw[:, 3].copy()
    # guard against ~zero tap3 (ratios r_k = w_k/w3)
    tiny = np.abs(w3) < 1e-5
    w3[tiny] = np.where(w3[tiny] < 0, -1e-5, 1e-5)
    r = [cw[:, k] / w3 for k in range(3)]

    # in_proj xc weights pre-scaled by tap3 per output channel
    wc_f = np.asarray(in_proj_w[:_DI], f32) * w3[:, None]
    wc_ = np.ascontiguousarray(wc_f.T.reshape(4, 128, _DI)).astype(f16)
    wz_ = np.ascontiguousarray(
        np.asarray(in_proj_w[_DI:], f32).T.reshape(4, 128, _DI)).astype(f16)
    xpm = np.zeros((_DI, 66), f32)
    xpm[:, :32] = np.asarray(x_proj_w, f32)[:32].T
    xpm[:, 32] = np.asarray(x_proj_w, f32)[32]
    xpm[:, 64] = np.asarray(x_proj_w, f32)[33]
    xp_ = np.ascontiguousarray(xpm.reshape(8, 128, 66)).astype(f16)
    dtp_ = np.concatenate(
        [np.asarray(dt_proj_w, f32).T,
         np.asarray(dt_proj_b, f32)[None, :]], axis=0).astype(f16)  # [33,1024]
    opw_ = np.ascontiguousarray(
        np.asarray(out_proj_w, f32).T.reshape(8, 128, _DM)).astype(f16)
    l0_ = np.ascontiguousarray(
        np.asarray(lin0_w, f32).T.reshape(4, 128, _HID)).astype(f16)
    l1_ = np.ascontiguousarray(
        np.asarray(lin1_w, f32).T.reshape(8, 128, _HID)).astype(f16)
    l2_ = np.ascontiguousarray(
        np.asarray(lin2_w, f32).T.reshape(8, 128, _OUT)).astype(f16)

    s0 = np.asarray(bn0_g, f32) / np.sqrt(np.asarray(bn0_v, f32) + _EPS)
    c0 = s0 * (np.asarray(lin0_b, f32) - np.asarray(bn0_m, f32)) + np.asarray(bn0_b, f32)
    s1 = np.asarray(bn1_g, f32) / np.sqrt(np.asarray(bn1_v, f32) + _EPS)
    c1 = s1 * (np.asarray(lin1_b, f32) - np.asarray(bn1_m, f32)) + np.asarray(bn1_b, f32)
    A = -np.exp(np.asarray(A_log, f32)[:, 0])

    vecs = np.stack(
        [pack_vec(r[0]), pack_vec(r[1]), pack_vec(r[2]),
         pack_vec(-r[0] * cb), pack_vec(-r[1] * cb), pack_vec(-r[2] * cb),
         pack_vec(cb), pack_vec(A), pack_vec(D_skip),
         pack_vec(s0), pack_vec(c0), pack_vec(s1), pack_vec(c1)],
        axis=2)  # [128, 8, 13]
    vecs = np.ascontiguousarray(vecs, f32)
    l2b_ = np.ascontiguousarray(np.asarray(lin2_b, f32)[:, None])

    shared = dict(wc=wc_, wz=wz_, xp=xp_, dtp=dtp_, opw=opw_, l0=l0_, l1=l1_,
                  l2=l2_, vecs=vecs, l2b=l2b_)

    n_shards = np.asarray(x).shape[0] // NB
    in_maps = []
    for c in range(n_shards):
        xs_ = np.asarray(x[c * NB:(c + 1) * NB], f32).reshape(NB * _L, _DM)
        xT = np.ascontiguousarray(xs_.T).astype(f16)
        in_maps.append(dict(shared, xT=xT))
    return in_maps


def kernel(**inputs):
    from concourse.bass_utils import run_bass_kernel_spmd

    NB = _B // _NC
    TB = 1024
    key = (NB, TB)
    if key not in _BUILD_CACHE:
        _BUILD_CACHE[key] = _build(NB, TB)
    nc = _BUILD_CACHE[key]

    in_maps = _prep_inputs(NB=NB, **inputs)
    res = run_bass_kernel_spmd(nc, in_maps, core_ids=list(range(_NC)))
    out = np.concatenate([r["out"] for r in res.results], axis=0)
    return out.astype(np.float32)
